# revision 1
# baseline (speedup 1.0000x reference)
"""Trainium2 Bass kernel for nn_AttentionHead (B=4, S=2048, H=D=1024, 8 cores).

Reference semantics (fp32):
    q = x @ Wq.T; k = x @ Wk.T; v = x @ Wv.T          (per batch b)
    kT = k.reshape(b, d, s)                            (raw reshape, NOT transpose)
    scores = q @ kT / sqrt(d)
    attn = softmax(scores, axis=0)                     (softmax over BATCH)
    attn_masked = where(tril(s, s), attn, 1e-9)
    out = attn_masked @ v

Sharding: every core computes k/v for a contiguous 256-row sequence shard and
the shards are exchanged with per-batch AllGathers; the attention map rows are
sharded as two 128-row blocks per core (blocks c and 15-c).  The batch-softmax
couples batches at identical (i, j), so all 4 batches of a given attention-map
tile live on one core.  Scores are built transposed ([j, i]) so the attn @ v
matmul needs no on-chip transpose.  The causal mask (and the 1e-9 fill) is
applied from host-precomputed per-core mask tensors.
"""

import numpy as np

B, S, H, D = 4, 2048, 1024, 1024
R = 8                  # cores
SL = S // R            # kv shard rows per core (contiguous)
IB = 128               # i block height
NBLK = 16              # 16 blocks of 128 rows
NJT = S // IB          # 16 j tiles of 128
ILOC = 2 * IB          # local q rows per core

_CACHE = {}


def _blocks(c):
    return (c, NBLK - 1 - c)


def _build_program():
    from contextlib import ExitStack

    import concourse.bacc as bacc
    import concourse.mybir as mybir
    from concourse import tile

    f32 = mybir.dt.float32
    nc = bacc.Bacc("TRN2", target_bir_lowering=False, debug=False, num_devices=R)

    xt_q = nc.dram_tensor("xt_q", [B, H, ILOC], f32, kind="ExternalInput").ap()
    xt_kv = nc.dram_tensor("xt_kv", [B, H, SL], f32, kind="ExternalInput").ap()
    wqt = nc.dram_tensor("wqt", [H, D], f32, kind="ExternalInput").ap()
    wkt = nc.dram_tensor("wkt", [H, D], f32, kind="ExternalInput").ap()
    wvt = nc.dram_tensor("wvt", [H, D], f32, kind="ExternalInput").ap()
    m1 = nc.dram_tensor("m1", [NJT, IB, ILOC], f32, kind="ExternalInput").ap()
    m2 = nc.dram_tensor("m2", [NJT, IB, ILOC], f32, kind="ExternalInput").ap()
    out_loc = nc.dram_tensor("out_loc", [B, ILOC, D], f32, kind="ExternalOutput").ap()

    with tile.TileContext(nc) as tc, ExitStack() as ctx:
        dram = ctx.enter_context(tc.tile_pool(name="dram", bufs=1, space="DRAM"))
        ag_in = dram.tile([B, 2, SL, D], f32)
        ag_out = [dram.tile([R, 2, SL, D], f32, name=f"ag_out{b}") for b in range(B)]

        # ---------------- KV projection + per-batch AllGather ----------------
        with tc.tile_pool(name="wkv", bufs=1) as wpool, \
             tc.tile_pool(name="xkv", bufs=2) as xpool, \
             tc.tile_pool(name="kvsb", bufs=3) as kvpool, \
             tc.tile_pool(name="pskv", bufs=2, space="PSUM") as pskv:
            wk_sb = wpool.tile([128, 8, D], f32, tag="wk")
            wv_sb = wpool.tile([128, 8, D], f32, tag="wv")
            nc.sync.dma_start(wk_sb[:], wkt.rearrange("(t p) d -> p t d", p=128))
            nc.sync.dma_start(wv_sb[:], wvt.rearrange("(t p) d -> p t d", p=128))
            for b in range(B):
                xkv_sb = xpool.tile([128, 8, SL], f32, tag="xkv")
                nc.sync.dma_start(
                    xkv_sb[:], xt_kv[b].rearrange("(t p) s -> p t s", p=128)
                )
                for kv, w_sb in ((0, wk_sb), (1, wv_sb)):
                    for st in range(SL // 128):
                        for dblk in range(D // 512):
                            ps = pskv.tile([128, 512], f32, tag="pskv")
                            for ht in range(8):
                                nc.tensor.matmul(
                                    ps[:],
                                    xkv_sb[:, ht, st * 128:(st + 1) * 128],
                                    w_sb[:, ht, dblk * 512:(dblk + 1) * 512],
                                    start=(ht == 0),
                                    stop=(ht == 7),
                                )
                            sb = kvpool.tile([128, 512], f32, tag="kvsb")
                            nc.vector.tensor_copy(sb[:], ps[:])
                            nc.sync.dma_start(
                                ag_in[b, kv, st * 128:(st + 1) * 128,
                                      dblk * 512:(dblk + 1) * 512],
                                sb[:],
                            )
                nc.gpsimd.collective_compute(
                    "AllGather",
                    mybir.AluOpType.bypass,
                    replica_groups=[list(range(R))],
                    ins=[ag_in[b]],
                    outs=[ag_out[b].opt()],
                )

        # ---------------- Q projection (overlaps the AllGathers) -------------
        with tc.tile_pool(name="qtpool", bufs=4) as qtpool:
            with tc.tile_pool(name="wq", bufs=1) as wqpool, \
                 tc.tile_pool(name="xq", bufs=2) as xqpool, \
                 tc.tile_pool(name="psq", bufs=2, space="PSUM") as psq:
                wq_sb = wqpool.tile([128, 8, D], f32, tag="wq")
                nc.sync.dma_start(wq_sb[:], wqt.rearrange("(t p) d -> p t d", p=128))
                qt_sb = []
                for b in range(B):
                    xq_sb = xqpool.tile([128, 8, ILOC], f32, tag="xq")
                    nc.sync.dma_start(
                        xq_sb[:], xt_q[b].rearrange("(t p) s -> p t s", p=128)
                    )
                    qt_b = qtpool.tile([128, 8, ILOC], f32, tag="qt")
                    qt_sb.append(qt_b)
                    for mt in range(8):
                        ps = psq.tile([128, ILOC], f32, tag="psq")
                        for ht in range(8):
                            nc.tensor.matmul(
                                ps[:],
                                wq_sb[:, ht, mt * 128:(mt + 1) * 128],
                                xq_sb[:, ht, :],
                                start=(ht == 0),
                                stop=(ht == 7),
                            )
                        nc.vector.tensor_copy(qt_b[:, mt, :], ps[:])

            # ---------------- scores (transposed) + exp ----------------------
            with tc.tile_pool(name="epool", bufs=4 * NJT) as epool, \
                 tc.tile_pool(name="ktpool", bufs=16) as ktpool, \
                 tc.tile_pool(name="pss", bufs=4, space="PSUM") as pss:
                e_tiles = [[None] * NJT for _ in range(B)]
                for b in range(B):
                    for jtg in range(4):          # groups of 4 j-tiles
                        jh, chalf = jtg // 2, jtg % 2
                        kts = []
                        for mt in range(8):
                            kt = ktpool.tile([128, 512], f32, tag="kt")
                            src = ag_out[b][mt, 0].rearrange(
                                "(p two) d -> two p d", two=2
                            )[jh, :, chalf * 512:(chalf + 1) * 512]
                            nc.sync.dma_start(kt[:], src)
                            kts.append(kt)
                        for q in range(4):
                            jt = jtg * 4 + q
                            ps = pss.tile([128, ILOC], f32, tag="pss")
                            for mt in range(8):
                                nc.tensor.matmul(
                                    ps[:],
                                    kts[mt][:, q * 128:(q + 1) * 128],
                                    qt_sb[b][:, mt, :],
                                    start=(mt == 0),
                                    stop=(mt == 7),
                                )
                            e = epool.tile([IB, ILOC], f32, tag="e")
                            nc.scalar.activation(
                                e[:], ps[:], mybir.ActivationFunctionType.Exp,
                                scale=float(1.0 / np.sqrt(D)),
                            )
                            e_tiles[b][jt] = e

                # ---------------- softmax over batch + masking ---------------
                with tc.tile_pool(name="smx", bufs=3) as smx, \
                     tc.tile_pool(name="mpool", bufs=4) as mpool:
                    for jt in range(NJT):
                        m1_sb = mpool.tile([IB, ILOC], f32, tag="m1")
                        m2_sb = mpool.tile([IB, ILOC], f32, tag="m2")
                        nc.sync.dma_start(m1_sb[:], m1[jt])
                        nc.sync.dma_start(m2_sb[:], m2[jt])
                        den = smx.tile([IB, ILOC], f32, tag="den")
                        nc.vector.tensor_add(
                            den[:], e_tiles[0][jt][:], e_tiles[1][jt][:]
                        )
                        nc.vector.tensor_add(den[:], den[:], e_tiles[2][jt][:])
                        nc.vector.tensor_add(den[:], den[:], e_tiles[3][jt][:])
                        rm = smx.tile([IB, ILOC], f32, tag="rm")
                        nc.vector.reciprocal(rm[:], den[:])
                        nc.vector.tensor_mul(rm[:], rm[:], m1_sb[:])
                        for b in range(B):
                            t = smx.tile([IB, ILOC], f32, tag="tmp")
                            nc.vector.tensor_mul(t[:], e_tiles[b][jt][:], rm[:])
                            nc.vector.tensor_add(
                                e_tiles[b][jt][:], t[:], m2_sb[:]
                            )

                # ---------------- attn @ v -----------------------------------
                with tc.tile_pool(name="vpool", bufs=4) as vpool, \
                     tc.tile_pool(name="opool", bufs=3) as opool, \
                     tc.tile_pool(name="psv", bufs=2, space="PSUM") as psv:
                    for b in range(B):
                        for nblk in range(D // 512):
                            ps0 = psv.tile([128, 512], f32, tag="pv0")
                            ps1 = psv.tile([128, 512], f32, tag="pv1")
                            for jt in range(NJT):
                                vt = vpool.tile([128, 512], f32, tag="vt")
                                nc.sync.dma_start(
                                    vt[:],
                                    ag_out[b][jt // 2, 1,
                                              (jt % 2) * 128:(jt % 2 + 1) * 128,
                                              nblk * 512:(nblk + 1) * 512],
                                )
                                for ih, ps in ((0, ps0), (1, ps1)):
                                    nc.tensor.matmul(
                                        ps[:],
                                        e_tiles[b][jt][:, ih * 128:(ih + 1) * 128],
                                        vt[:],
                                        start=(jt == 0),
                                        stop=(jt == NJT - 1),
                                    )
                            for ih, ps in ((0, ps0), (1, ps1)):
                                osb = opool.tile([128, 512], f32, tag="osb")
                                nc.vector.tensor_copy(osb[:], ps[:])
                                nc.sync.dma_start(
                                    out_loc[b, ih * 128:(ih + 1) * 128,
                                            nblk * 512:(nblk + 1) * 512],
                                    osb[:],
                                )

    nc.compile()
    return nc


def _host_inputs(x, Wq, Wk, Wv):
    x = np.ascontiguousarray(x, dtype=np.float32)
    wqt = np.ascontiguousarray(Wq.T, dtype=np.float32)
    wkt = np.ascontiguousarray(Wk.T, dtype=np.float32)
    wvt = np.ascontiguousarray(Wv.T, dtype=np.float32)

    in_maps = []
    for c in range(R):
        blo, bhi = _blocks(c)
        rows = np.r_[blo * IB:(blo + 1) * IB, bhi * IB:(bhi + 1) * IB]
        xt_q = np.ascontiguousarray(x[:, rows, :].transpose(0, 2, 1))
        xt_kv = np.ascontiguousarray(
            x[:, c * SL:(c + 1) * SL, :].transpose(0, 2, 1)
        )
        gi = rows[None, None, :]                       # global i (1,1,ILOC)
        jj = (np.arange(NJT)[:, None, None] * IB
              + np.arange(IB)[None, :, None])          # global j (NJT,IB,1)
        m1 = (jj <= gi).astype(np.float32)
        m2 = ((1.0 - m1) * np.float32(1e-9)).astype(np.float32)
        in_maps.append({
            "xt_q": xt_q, "xt_kv": xt_kv,
            "wqt": wqt, "wkt": wkt, "wvt": wvt,
            "m1": np.ascontiguousarray(m1), "m2": np.ascontiguousarray(m2),
        })
    return in_maps


def kernel(x, Wq, Wk, Wv):
    from concourse.bass_utils import run_bass_kernel_spmd

    if "nc" not in _CACHE:
        _CACHE["nc"] = _build_program()
    nc = _CACHE["nc"]

    in_maps = _host_inputs(x, Wq, Wk, Wv)
    res = run_bass_kernel_spmd(nc, in_maps, list(range(R)))

    out = np.empty((B, S, D), dtype=np.float32)
    for c in range(R):
        blo, bhi = _blocks(c)
        ol = res.results[c]["out_loc"]
        out[:, blo * IB:(blo + 1) * IB, :] = ol[:, :IB, :]
        out[:, bhi * IB:(bhi + 1) * IB, :] = ol[:, IB:, :]
    return out


if __name__ == "__main__":
    rng = np.random.default_rng(0)
    x = rng.standard_normal((B, S, H), dtype=np.float32)
    Wq = rng.standard_normal((D, H), dtype=np.float32) / np.sqrt(H)
    Wk = rng.standard_normal((D, H), dtype=np.float32) / np.sqrt(H)
    Wv = rng.standard_normal((D, H), dtype=np.float32) / np.sqrt(H)
    o = kernel(x, Wq, Wk, Wv)
    print("kernel output", o.shape, o.dtype, float(np.abs(o).max()))
